# revision 11
# baseline (speedup 1.0000x reference)
"""LoRA multi-head attention on 8 Trainium2 NeuronCores — v2.

Problem: B=4, S=2048, D=1024, H=16, HD=64, RANK=16 LoRA on q/v.
Sharding: core c handles batch c//2 and heads (c%2)*8 .. (c%2)*8+8.
Per-core output partials (the two head-halves of a batch) are summed on
the host along with the output bias.  No device collectives.

v2 changes vs the original baseline:
  - LoRA folded into the dense projection weights on the host
    (W' = W + scaling*(B@A)): removes the xa stage and per-tile LoRA-up
    matmuls entirely.
  - Input DMAs split across both HWDGE rings (SP + Activation) with
    xT/wq/wk first so the attention pipeline starts as early as possible.
  - ACT exp table preloaded at t=0 via a dummy activation.
  - q/k for pair 0 computed per-sq-block so block (0,0) starts sooner.
  - v projection emitted in per-pair column slices, dripped into the
    attention blocks just before first use; q/k for later pairs dripped
    with finer (q-only / k-only) granularity.
  - ctx matmuls lag the exp stream by a configurable number of chunks so
    late v slices don't stall the PE queue ahead of the scores matmuls.
  - Optional: a fraction of the softmax exp chunks computed on the DVE
    via a Schraudolph bf16 fast-exp (tensor_scalar f32->int16, bitcast
    to bf16), offloading the bottleneck ACT engine.
  - Output written as bf16 (halves the out DMA); host accumulates in f32.

Per-core dataflow (all matmul inputs bf16, PSUM f32):
  xT[D,S] -> qT/kT[oc,S] (transposed proj, LoRA + 1/sqrt(HD) folded in)
          -> v[S,oc] slices with a ones column per head
  scoresT[sk,sq] = kT.T-chunks x qT (2 heads row-tiled in the 128-wide PE)
  expT = Exp(scoresT + mask[sk]) on ACT (or DVE fast-exp)
  ctx_aug[65,sq] = v_aug.T x expT   (row 64 = softmax denominator)
  ctxT = ctx_aug[0:64] * bcast(1/denom)   (PE K=1 broadcast matmul)
  outT-partial[sq, D] = ctxT-chunks x Wo.T-chunks
"""

import math
from contextlib import ExitStack

import numpy as np
import ml_dtypes

import concourse.bass as bass
import concourse.mybir as mybir
import concourse.tile as tile
from concourse import bacc
from concourse.bass_utils import run_bass_kernel_spmd

F32 = mybir.dt.float32
BF16 = mybir.dt.bfloat16
I16 = mybir.dt.int16
NPBF16 = ml_dtypes.bfloat16

B, S, D = 4, 2048, 1024
H, HD = 16, 64
RANK = 16
SCALING = 32.0 / RANK  # 2.0
NCORES = 8
HPC = H // 2        # heads per core = 8
OC = HPC * HD       # output cols per core = 512
NPAIR = HPC // 2    # head pairs per core = 4
KC = D // 128       # 8 contraction chunks
SQB = 512           # sq block
NSQB = S // SQB     # 4
NSK = S // 128      # 16 sk chunks

FE_A = 128.0 / math.log(2.0)   # fast-exp scale
FE_C = 7.0                     # fast-exp additive tuning constant

_NC_CACHE = {}


def _default_fast_exp():
    """(pair, sqb) -> set of chunk indices computed on DVE instead of ACT.

    Placed in blocks with little PE drip work (late pairs) where the ACT
    exp stream is the critical path and the DVE is mostly idle.
    """
    fe = {}
    for blk in ((2, 0), (2, 1), (2, 2), (2, 3), (3, 0), (3, 1), (3, 2)):
        fe[blk] = {1, 3, 5, 7, 9, 11, 13}
    return fe


def _build_nc(loop_n=None, fast_exp=None, ctx_lag=4):
    """Build the (SPMD, per-core) Bass/Tile program once.

    fast_exp: dict (pair, sqb) -> iterable of chunk idxs to run on DVE.
    ctx_lag: how many stream steps the ctx matmuls trail the exp stream.
    """
    if fast_exp is None:
        fast_exp = _default_fast_exp()
    nc = bacc.Bacc("TRN2", target_bir_lowering=False, debug=False)

    xT_d = nc.dram_tensor("xT", [D, S], BF16, kind="ExternalInput")
    wq_d = nc.dram_tensor("wq", [D, OC], BF16, kind="ExternalInput")
    wk_d = nc.dram_tensor("wk", [D, OC], BF16, kind="ExternalInput")
    wv_d = nc.dram_tensor("wv", [D, OC], BF16, kind="ExternalInput")
    wo_d = nc.dram_tensor("wo", [OC, D], BF16, kind="ExternalInput")
    mask_d = nc.dram_tensor("mask", [128, NSK], F32, kind="ExternalInput")
    maskf_d = nc.dram_tensor("maskf", [128, NSK], F32, kind="ExternalInput")
    out_d = nc.dram_tensor("out", [S, D], BF16, kind="ExternalOutput")

    with tile.TileContext(nc) as tc, ExitStack() as ctx:
        consts = ctx.enter_context(tc.tile_pool(name="consts", bufs=1))
        expp = ctx.enter_context(tc.tile_pool(name="expp", bufs=8))
        dnrp = ctx.enter_context(tc.tile_pool(name="dnrp", bufs=2))
        bcp = ctx.enter_context(tc.tile_pool(name="bcp", bufs=2))
        tmbp = ctx.enter_context(tc.tile_pool(name="tmbp", bufs=2))
        outp = ctx.enter_context(tc.tile_pool(name="outp", bufs=4))
        ps_sc = ctx.enter_context(tc.tile_pool(name="ps_sc", bufs=2, space="PSUM"))
        ps_ctx = ctx.enter_context(tc.tile_pool(name="ps_ctx", bufs=1, space="PSUM"))
        ps_mm = ctx.enter_context(tc.tile_pool(name="ps_mm", bufs=2, space="PSUM"))

        # ---- persistent SBUF tiles --------------------------------------
        xT = consts.tile([128, KC, S], BF16, tag="xT")
        wq = consts.tile([128, KC, OC], BF16, tag="wq")
        wk = consts.tile([128, KC, OC], BF16, tag="wk")
        wv = consts.tile([128, KC, OC], BF16, tag="wv")
        wo = consts.tile([128, NPAIR, D], BF16, tag="wo")
        mask = consts.tile([128, NSK], F32, tag="mask")
        maskf = consts.tile([128, NSK], F32, tag="maskf")
        ones = consts.tile([128, 64], F32, tag="ones")
        qT = consts.tile([128, NPAIR, S], BF16, tag="qT")
        kT = consts.tile([128, NPAIR, S], BF16, tag="kT")
        vsb = consts.tile([128, NSK, HPC, HD + 1], BF16, tag="vsb")
        ctxT = consts.tile([128, NPAIR, S], BF16, tag="ctxT")
        warm = consts.tile([1, 8], F32, tag="warm")

        def emit():
            # ---- input DMAs, split across the two HWDGE rings ----------
            # ring A (SP): xT (needed by everything) then wv, wo.
            # ring B (ACT): wq, wk (gate the first scores), masks.
            for c in range(KC):
                nc.sync.dma_start(out=xT[:, c, :], in_=xT_d[c * 128:(c + 1) * 128, :])
            for c in range(KC):
                nc.scalar.dma_start(out=wq[:, c, :], in_=wq_d[c * 128:(c + 1) * 128, :])
                nc.scalar.dma_start(out=wk[:, c, :], in_=wk_d[c * 128:(c + 1) * 128, :])
            nc.scalar.dma_start(out=mask[:, :], in_=mask_d[:, :])
            nc.scalar.dma_start(out=maskf[:, :], in_=maskf_d[:, :])
            for c in range(KC):
                nc.sync.dma_start(out=wv[:, c, :], in_=wv_d[c * 128:(c + 1) * 128, :])
            for p in range(NPAIR):
                nc.sync.dma_start(out=wo[:, p, :], in_=wo_d[p * 128:(p + 1) * 128, :])
            nc.vector.memset(ones[:, :], 1.0)
            nc.vector.memset(vsb[:, :, :, HD:HD + 1], 1.0)
            # ACT exp-table preload while DMAs run
            nc.vector.memset(warm[:, :], 0.0)
            nc.scalar.activation(
                out=warm[:, :], in_=warm[:, :],
                func=mybir.ActivationFunctionType.Exp, scale=1.0)

            def emit_v_slice(sc, plo, phi):
                # v projection for s-chunk sc, head-pairs [plo, phi)
                lo, hi = plo * 128, phi * 128
                ps = ps_mm.tile([128, 512], F32, tag="ps", name=f"vp{sc}_{plo}")
                w = hi - lo
                for c in range(KC):
                    nc.tensor.matmul(
                        ps[:, 0:w], xT[:, c, sc * 128:(sc + 1) * 128],
                        wv[:, c, lo:hi],
                        start=(c == 0), stop=(c == KC - 1))
                nc.vector.tensor_copy(
                    vsb[:, sc, 2 * plo:2 * phi, 0:HD],
                    ps[:, 0:w].rearrange("p (h d) -> p h d", d=HD))

            def emit_q_proj(p, sqb):
                sq = slice(sqb * SQB, (sqb + 1) * SQB)
                ps = ps_mm.tile([128, SQB], F32, tag="ps", name=f"qp{p}_{sqb}")
                for c in range(KC):
                    nc.tensor.matmul(
                        ps[:, :], wq[:, c, p * 128:(p + 1) * 128], xT[:, c, sq],
                        start=(c == 0), stop=(c == KC - 1))
                nc.vector.tensor_copy(qT[:, p, sq], ps[:, :])

            def emit_k_proj(p, sqb):
                sq = slice(sqb * SQB, (sqb + 1) * SQB)
                ps = ps_mm.tile([128, SQB], F32, tag="ps", name=f"kp{p}_{sqb}")
                for c in range(KC):
                    nc.tensor.matmul(
                        ps[:, :], wk[:, c, p * 128:(p + 1) * 128], xT[:, c, sq],
                        start=(c == 0), stop=(c == KC - 1))
                nc.vector.tensor_copy(kT[:, p, sq], ps[:, :])

            # minimal preamble: q(0,0) + k(0,0) gate the first scores; the
            # remaining k(0,*) are dripped early inside the stream (scores
            # chunk 4j only needs kT up to sq-block j).
            emit_q_proj(0, 0)
            emit_k_proj(0, 0)

            # ---- global chunk stream -----------------------------------
            # All 16 blocks' score/exp chunks form one stream of 256 steps;
            # ctx matmuls trail by `lag` steps, so block boundaries pipeline
            # (the previous block's ctx tail and epilogue overlap the next
            # block's score/exp head).  PE drip work (projections, v
            # slices, out-proj) is scheduled at explicit stream indices.
            lag = ctx_lag
            fill = {}

            def drip(gi, thunk):
                fill.setdefault(gi, []).append(thunk)

            # k for pair 0, sq-blocks 1-3: needed by scores chunk 4j
            drip(1, lambda: emit_k_proj(0, 1))
            drip(4, lambda: emit_k_proj(0, 2))
            drip(7, lambda: emit_k_proj(0, 3))
            # v slices, pair 0 (needed at ctx(sc) = stream sc+lag; start at
            # +3 so the wv DMA has landed before the first slice)
            for sc in range(NSK):
                drip(sc + 3, lambda sc=sc: emit_v_slice(sc, 0, 1))
            # q for pair 0, sq-blocks 1-3: needed at stream 16j
            drip(11, lambda: emit_q_proj(0, 1))
            drip(26, lambda: emit_q_proj(0, 2))
            drip(42, lambda: emit_q_proj(0, 3))
            # v slices for pairs 1+2 (256-col), needed from stream 64+lag
            for sc in range(NSK):
                drip(18 + 2 * sc, lambda sc=sc: emit_v_slice(sc, 1, 3))
            # q/k for pair 1: k needed by 64+4j, q(1,j) by 64+16j
            drip(46, lambda: emit_q_proj(1, 0))
            drip(50, lambda: emit_k_proj(1, 0))
            drip(54, lambda: emit_k_proj(1, 1))
            drip(58, lambda: emit_k_proj(1, 2))
            drip(61, lambda: emit_k_proj(1, 3))
            drip(74, lambda: emit_q_proj(1, 1))
            drip(90, lambda: emit_q_proj(1, 2))
            # v slices for pair 3, needed from stream 192+lag
            for sc in range(NSK):
                drip(66 + 2 * sc, lambda sc=sc: emit_v_slice(sc, 3, 4))
            drip(104, lambda: emit_q_proj(1, 3))
            # q/k for pair 2
            drip(108, lambda: emit_q_proj(2, 0))
            drip(112, lambda: emit_k_proj(2, 0))
            drip(116, lambda: emit_k_proj(2, 1))
            drip(120, lambda: emit_k_proj(2, 2))
            drip(124, lambda: emit_k_proj(2, 3))
            drip(138, lambda: emit_q_proj(2, 1))
            drip(154, lambda: emit_q_proj(2, 2))
            drip(168, lambda: emit_q_proj(2, 3))
            # q/k for pair 3
            drip(172, lambda: emit_q_proj(3, 0))
            drip(176, lambda: emit_k_proj(3, 0))
            drip(180, lambda: emit_k_proj(3, 1))
            drip(184, lambda: emit_k_proj(3, 2))
            drip(188, lambda: emit_k_proj(3, 3))
            drip(202, lambda: emit_q_proj(3, 1))
            drip(218, lambda: emit_q_proj(3, 2))
            drip(232, lambda: emit_q_proj(3, 3))

            def emit_outproj_sc2(sc2):
                # pp-outer so each ctxT stationary is loaded once for both
                # 512-wide output halves
                s2 = slice(sc2 * 128, (sc2 + 1) * 128)
                po0 = ps_mm.tile([128, 512], F32, tag="ps", name=f"op{sc2}_0")
                po1 = ps_mm.tile([128, 512], F32, tag="ps", name=f"op{sc2}_1")
                for pp in range(NPAIR):
                    nc.tensor.matmul(
                        po0[:, :], ctxT[:, pp, s2], wo[:, pp, 0:512],
                        start=(pp == 0), stop=(pp == NPAIR - 1))
                    nc.tensor.matmul(
                        po1[:, :], ctxT[:, pp, s2], wo[:, pp, 512:1024],
                        start=(pp == 0), stop=(pp == NPAIR - 1))
                for oh, po in ((0, po0), (1, po1)):
                    ot = outp.tile([128, 512], BF16, tag="ot")
                    nc.vector.tensor_copy(ot[:, :], po[:, :])
                    eng = nc.sync if oh == 0 else nc.scalar
                    eng.dma_start(
                        out=out_d[s2, oh * 512:(oh + 1) * 512], in_=ot[:, :])

            # out-proj: block (3, sqb)'s four 128-row slices, spread over
            # the stream right after that block's epilogue
            for sqb in range(NSQB):
                epi = 16 * (3 * NSQB + sqb) + NSK - 1 + lag  # epilogue step
                for k2 in range(SQB // 128):
                    drip(epi + 1 + k2,
                         lambda sc2=sqb * (SQB // 128) + k2: emit_outproj_sc2(sc2))

            def epilogue(p, sqb, cA, cB):
                sq = slice(sqb * SQB, (sqb + 1) * SQB)
                # 1/denom, broadcast over 64 partitions, scale.  The fast NR
                # reciprocal miscomputes on partition-sliced inputs, so feed
                # it the FULL [65, SQB] ctx psum tile: rows 0-63 produce
                # garbage nothing reads; row 64 is the denominator row.
                dnr = dnrp.tile([65, 2 * SQB], F32, tag="dnr")
                nc.vector.reciprocal_approx_fast(
                    out=dnr[0:65, 0:SQB], in_=cA[0:65, :])
                nc.vector.reciprocal_approx_fast(
                    out=dnr[0:65, SQB:2 * SQB], in_=cB[0:65, :])
                bcA = ps_mm.tile([64, SQB], F32, tag="ps", name="bcA")
                nc.tensor.matmul(
                    bcA[:, :], ones[64:65, 0:64], dnr[64:65, 0:SQB],
                    start=True, stop=True, tile_position=(64, 0))
                bcB = ps_mm.tile([64, SQB], F32, tag="ps", name="bcB")
                nc.tensor.matmul(
                    bcB[:, :], ones[64:65, 0:64], dnr[64:65, SQB:2 * SQB],
                    start=True, stop=True, tile_position=(64, 0))
                bc = bcp.tile([64, 2 * SQB], F32, tag="bc")
                nc.vector.tensor_copy(bc[:, 0:SQB], bcA[:, :])
                nc.vector.tensor_copy(bc[:, SQB:2 * SQB], bcB[:, :])
                nc.vector.tensor_mul(
                    ctxT[0:64, p, sq], cA[0:64, :], bc[:, 0:SQB])
                tmb = tmbp.tile([64, SQB], BF16, tag="tmb")
                nc.vector.tensor_mul(tmb[:, :], cB[0:64, :], bc[:, SQB:2 * SQB])
                # partition shift (rows 0-63 -> 64-127) via DMA
                nc.sync.dma_start(out=ctxT[64:128, p, sq], in_=tmb[:, :])

            NBLK = NPAIR * NSQB
            exps = [None] * (NBLK * NSK)
            cur = {}  # block -> (cA, cB)

            def emit_ctx(gj):
                b, j = gj // NSK, gj % NSK
                p = b // NSQB
                if j == 0:
                    cur[b] = (ps_ctx.tile([HD + 1, SQB], F32, tag="cA",
                                          name=f"cA{b}"),
                              ps_ctx.tile([HD + 1, SQB], F32, tag="cB",
                                          name=f"cB{b}"))
                cA, cB = cur[b]
                exl = exps[gj]
                nc.tensor.matmul(
                    cA[:, :], vsb[:, j, 2 * p, :], exl[:, 0:SQB],
                    start=(j == 0), stop=(j == NSK - 1))
                nc.tensor.matmul(
                    cB[:, :], vsb[:, j, 2 * p + 1, :], exl[:, SQB:2 * SQB],
                    start=(j == 0), stop=(j == NSK - 1))
                if j == NSK - 1:
                    epilogue(p, b % NSQB, cA, cB)
                    del cur[b]

            last_gi = max(NBLK * NSK + lag - 1, max(fill))
            for gi in range(last_gi + 1):
                if gi < NBLK * NSK:
                    b, i = gi // NSK, gi % NSK
                    p, sqb = b // NSQB, b % NSQB
                    sq = slice(sqb * SQB, (sqb + 1) * SQB)
                    sk = slice(i * 128, (i + 1) * 128)
                    sc_ps = ps_sc.tile([128, 2 * SQB], F32, tag="sc")
                    # two heads packed in the PE rows (K=64 each)
                    nc.tensor.matmul(
                        sc_ps[:, 0:SQB], kT[0:64, p, sk], qT[0:64, p, sq],
                        start=True, stop=True, tile_position=(0, 0))
                    nc.tensor.matmul(
                        sc_ps[:, SQB:2 * SQB], kT[64:128, p, sk],
                        qT[64:128, p, sq],
                        start=True, stop=True, tile_position=(64, 0))
                    ex = expp.tile([128, 2 * SQB], BF16, tag="ex")
                    if i in fast_exp.get((p, sqb), ()):
                        # DVE fast-exp: Schraudolph in bf16 bit space
                        nc.vector.tensor_scalar(
                            out=ex[:, :].bitcast(I16), in0=sc_ps[:, :],
                            scalar1=FE_A, scalar2=maskf[:, i:i + 1],
                            op0=mybir.AluOpType.mult,
                            op1=mybir.AluOpType.add)
                    else:
                        nc.scalar.activation(
                            out=ex[:, :], in_=sc_ps[:, :],
                            func=mybir.ActivationFunctionType.Exp,
                            bias=mask[:, i:i + 1], scale=1.0)
                    exps[gi] = ex
                for thunk in fill.get(gi, ()):
                    thunk()
                if lag <= gi < NBLK * NSK + lag:
                    emit_ctx(gi - lag)

        if loop_n is None:
            emit()
        else:
            with tc.For_i(0, loop_n, 1):
                emit()

    nc.compile()
    return nc


def _prep_core_inputs(x, am, Wq, Aq, Bq, Wk, Wv, Av, Bv, Wo):
    """Host-side shard + layout prep. Returns the 8 per-core input dicts."""
    s = 1.0 / math.sqrt(HD)
    # fold LoRA into the dense weights: q/v = x @ (W + scaling*B@A).T
    WqE = (Wq + SCALING * (Bq @ Aq)) * s
    WvE = Wv + SCALING * (Bv @ Av)
    wqT = np.ascontiguousarray(WqE.T).astype(NPBF16)           # [D, D]
    wkT = np.ascontiguousarray(Wk.T).astype(NPBF16)
    wvT = np.ascontiguousarray(WvE.T).astype(NPBF16)
    woT = np.ascontiguousarray(Wo.T).astype(NPBF16)            # [D, D]
    in_maps = []
    for core in range(NCORES):
        b, hh = core // 2, core % 2
        cs = slice(hh * OC, (hh + 1) * OC)
        xT = np.ascontiguousarray(x[b].T).astype(NPBF16)       # [D, S]
        m = np.ascontiguousarray(
            am[b, 0, 0, :].astype(np.float32).reshape(NSK, 128).T)  # [128, NSK]
        mf = np.ascontiguousarray(m * FE_A + (16256.0 - FE_C))
        in_maps.append({
            "xT": xT,
            "wq": np.ascontiguousarray(wqT[:, cs]),
            "wk": np.ascontiguousarray(wkT[:, cs]),
            "wv": np.ascontiguousarray(wvT[:, cs]),
            "wo": np.ascontiguousarray(woT[cs, :]),
            "mask": m,
            "maskf": mf,
        })
    return in_maps


def kernel(_trace=False, _trace_kwargs=None, **inputs):
    x = np.asarray(inputs["hidden_states"], dtype=np.float32)
    am = np.asarray(inputs["attention_mask"], dtype=np.float32)
    Wq = np.asarray(inputs["Wq"], dtype=np.float32)
    bq = np.asarray(inputs["bq"], dtype=np.float32)
    Aq = np.asarray(inputs["Aq"], dtype=np.float32)
    Bq = np.asarray(inputs["Bq"], dtype=np.float32)
    Wk = np.asarray(inputs["Wk"], dtype=np.float32)
    bk = np.asarray(inputs["bk"], dtype=np.float32)
    Wv = np.asarray(inputs["Wv"], dtype=np.float32)
    bv = np.asarray(inputs["bv"], dtype=np.float32)
    Av = np.asarray(inputs["Av"], dtype=np.float32)
    Bv = np.asarray(inputs["Bv"], dtype=np.float32)
    Wo = np.asarray(inputs["Wo"], dtype=np.float32)
    bo = np.asarray(inputs["bo"], dtype=np.float32)

    # Projection biases are all-zero in this problem's regime (asserted so
    # a violated assumption fails loudly rather than silently returning
    # wrong results).
    assert not bq.any() and not bk.any() and not bv.any(), (
        "non-zero projection biases not supported by this kernel build")

    if "nc" not in _NC_CACHE:
        _NC_CACHE["nc"] = _build_nc()
    nc = _NC_CACHE["nc"]

    in_maps = _prep_core_inputs(x, am, Wq, Aq, Bq, Wk, Wv, Av, Bv, Wo)
    res = run_bass_kernel_spmd(
        nc, in_maps, core_ids=list(range(NCORES)), trace=_trace,
        trace_kwargs=_trace_kwargs or {})
    outs = res.results

    out = np.empty((B, S, D), dtype=np.float32)
    for b in range(B):
        out[b] = (outs[2 * b]["out"].astype(np.float32)
                  + outs[2 * b + 1]["out"].astype(np.float32) + bo)
    if _trace:
        return out, res
    return out


# revision 26
# speedup vs baseline: 1.5315x; 1.5315x over previous
"""LoRA multi-head attention on 8 Trainium2 NeuronCores — v2.

Problem: B=4, S=2048, D=1024, H=16, HD=64, RANK=16 LoRA on q/v.
Sharding: core c handles batch c//2 and heads (c%2)*8 .. (c%2)*8+8.
Per-core output partials (the two head-halves of a batch) are summed on
the host along with the output bias.  No device collectives.

v2 changes vs the original baseline:
  - LoRA folded into the dense projection weights on the host
    (W' = W + scaling*(B@A)): removes the xa stage and per-tile LoRA-up
    matmuls entirely.
  - Input DMAs split across both HWDGE rings (SP + Activation) with
    xT/wq/wk first so the attention pipeline starts as early as possible.
  - ACT exp table preloaded at t=0 via a dummy activation.
  - q/k for pair 0 computed per-sq-block so block (0,0) starts sooner.
  - v projection emitted in per-pair column slices, dripped into the
    attention blocks just before first use; q/k for later pairs dripped
    with finer (q-only / k-only) granularity.
  - ctx matmuls lag the exp stream by a configurable number of chunks so
    late v slices don't stall the PE queue ahead of the scores matmuls.
  - Optional: a fraction of the softmax exp chunks computed on the DVE
    via a Schraudolph bf16 fast-exp (tensor_scalar f32->int16, bitcast
    to bf16), offloading the bottleneck ACT engine.
  - Output written as bf16 (halves the out DMA); host accumulates in f32.

Per-core dataflow (all matmul inputs bf16, PSUM f32):
  xT[D,S] -> qT/kT[oc,S] (transposed proj, LoRA + 1/sqrt(HD) folded in)
          -> v[S,oc] slices with a ones column per head
  scoresT[sk,sq] = kT.T-chunks x qT (2 heads row-tiled in the 128-wide PE)
  expT = Exp(scoresT + mask[sk]) on ACT (or DVE fast-exp)
  ctx_aug[65,sq] = v_aug.T x expT   (row 64 = softmax denominator)
  ctxT = ctx_aug[0:64] * bcast(1/denom)   (PE K=1 broadcast matmul)
  outT-partial[sq, D] = ctxT-chunks x Wo.T-chunks
"""

import math
from contextlib import ExitStack

import numpy as np
import ml_dtypes

import concourse.bass as bass
import concourse.mybir as mybir
import concourse.tile as tile
from concourse import bacc
from concourse.bass_utils import run_bass_kernel_spmd

F32 = mybir.dt.float32
BF16 = mybir.dt.bfloat16
I16 = mybir.dt.int16
NPBF16 = ml_dtypes.bfloat16

B, S, D = 4, 2048, 1024
H, HD = 16, 64
RANK = 16
SCALING = 32.0 / RANK  # 2.0
NCORES = 8
HPC = H // 2        # heads per core = 8
OC = HPC * HD       # output cols per core = 512
NPAIR = HPC // 2    # head pairs per core = 4
KC = D // 128       # 8 contraction chunks
SQB = 512           # sq block
NSQB = S // SQB     # 4
NSK = S // 128      # 16 sk chunks

FE_A = 128.0 / math.log(2.0)   # fast-exp scale
FE_C = 7.0                     # fast-exp additive tuning constant

_NC_CACHE = {}


def _default_fast_exp():
    """(pair, sqb) -> set of chunk indices computed on DVE instead of ACT.

    Placed in blocks with little PE drip work (late pairs) where the ACT
    exp stream is the critical path and the DVE is mostly idle.
    """
    fe = {}
    for blk in ((1, 0), (1, 1), (2, 0), (2, 1), (2, 2), (2, 3),
                (3, 0), (3, 1), (3, 2)):
        fe[blk] = {1, 3, 5, 7, 9, 11, 13}
    return fe


def _build_nc(loop_n=None, fast_exp=None, ctx_lag=4):
    """Build the (SPMD, per-core) Bass/Tile program once.

    fast_exp: dict (pair, sqb) -> iterable of chunk idxs to run on DVE.
    ctx_lag: how many stream steps the ctx matmuls trail the exp stream.
    """
    if fast_exp is None:
        fast_exp = _default_fast_exp()
    nc = bacc.Bacc("TRN2", target_bir_lowering=False, debug=False)

    xT_d = nc.dram_tensor("xT", [D, S], BF16, kind="ExternalInput")
    wq_d = nc.dram_tensor("wq", [D, OC], BF16, kind="ExternalInput")
    wk_d = nc.dram_tensor("wk", [D, OC], BF16, kind="ExternalInput")
    wv_d = nc.dram_tensor("wv", [D, OC], BF16, kind="ExternalInput")
    wo_d = nc.dram_tensor("wo", [OC, D], BF16, kind="ExternalInput")
    mask_d = nc.dram_tensor("mask", [128, NSK], F32, kind="ExternalInput")
    maskf_d = nc.dram_tensor("maskf", [128, NSK], F32, kind="ExternalInput")
    out_d = nc.dram_tensor("out", [S, D], BF16, kind="ExternalOutput")

    with tile.TileContext(nc) as tc, ExitStack() as ctx:
        consts = ctx.enter_context(tc.tile_pool(name="consts", bufs=1))
        expp = ctx.enter_context(tc.tile_pool(name="expp", bufs=8))
        dnrp = ctx.enter_context(tc.tile_pool(name="dnrp", bufs=2))
        bcp = ctx.enter_context(tc.tile_pool(name="bcp", bufs=2))
        tmbp = ctx.enter_context(tc.tile_pool(name="tmbp", bufs=2))
        outp = ctx.enter_context(tc.tile_pool(name="outp", bufs=4))
        ps_sc = ctx.enter_context(tc.tile_pool(name="ps_sc", bufs=2, space="PSUM"))
        ps_ctx = ctx.enter_context(tc.tile_pool(name="ps_ctx", bufs=1, space="PSUM"))
        ps_mm = ctx.enter_context(tc.tile_pool(name="ps_mm", bufs=2, space="PSUM"))

        # ---- persistent SBUF tiles --------------------------------------
        xT = consts.tile([128, KC, S], BF16, tag="xT")
        wq = consts.tile([128, KC, OC], BF16, tag="wq")
        wk = consts.tile([128, KC, OC], BF16, tag="wk")
        wv = consts.tile([128, KC, OC], BF16, tag="wv")
        wo = consts.tile([128, NPAIR, D], BF16, tag="wo")
        mask = consts.tile([128, NSK], F32, tag="mask")
        maskf = consts.tile([128, NSK], F32, tag="maskf")
        ones = consts.tile([128, 64], F32, tag="ones")
        qT = consts.tile([128, NPAIR, S], BF16, tag="qT")
        kT = consts.tile([128, NPAIR, S], BF16, tag="kT")
        vsb = consts.tile([128, NSK, HPC, HD + 1], BF16, tag="vsb")
        ctxT = consts.tile([128, NPAIR, S], BF16, tag="ctxT")
        warm = consts.tile([1, 8], F32, tag="warm")

        def emit():
            # ---- input DMAs, split across the two HWDGE rings ----------
            # xT gates the first q/k projections: split it across both
            # rings, then wq/wk (interleaved per chunk so early proj
            # matmuls can start), then wv, masks, wo.
            for c in range(0, KC, 2):
                nc.sync.dma_start(out=xT[:, c, :], in_=xT_d[c * 128:(c + 1) * 128, :])
                nc.scalar.dma_start(
                    out=xT[:, c + 1, :], in_=xT_d[(c + 1) * 128:(c + 2) * 128, :])
            for c in range(KC):
                nc.sync.dma_start(out=wq[:, c, :], in_=wq_d[c * 128:(c + 1) * 128, :])
                nc.scalar.dma_start(out=wk[:, c, :], in_=wk_d[c * 128:(c + 1) * 128, :])
            nc.scalar.dma_start(out=mask[:, :], in_=mask_d[:, :])
            nc.scalar.dma_start(out=maskf[:, :], in_=maskf_d[:, :])
            for c in range(KC):
                nc.sync.dma_start(out=wv[:, c, :], in_=wv_d[c * 128:(c + 1) * 128, :])
            for p in range(NPAIR):
                nc.sync.dma_start(out=wo[:, p, :], in_=wo_d[p * 128:(p + 1) * 128, :])
            nc.vector.memset(ones[:, :], 1.0)
            nc.vector.memset(vsb[:, :, :, HD:HD + 1], 1.0)
            # ACT exp-table preload while DMAs run
            nc.vector.memset(warm[:, :], 0.0)
            nc.scalar.activation(
                out=warm[:, :], in_=warm[:, :],
                func=mybir.ActivationFunctionType.Exp, scale=1.0)

            def emit_v_slice(sc, plo, phi):
                # v projection for s-chunk sc, head-pairs [plo, phi)
                lo, hi = plo * 128, phi * 128
                ps = ps_mm.tile([128, 512], F32, tag="ps", name=f"vp{sc}_{plo}")
                w = hi - lo
                for c in range(KC):
                    nc.tensor.matmul(
                        ps[:, 0:w], xT[:, c, sc * 128:(sc + 1) * 128],
                        wv[:, c, lo:hi],
                        start=(c == 0), stop=(c == KC - 1))
                nc.vector.tensor_copy(
                    vsb[:, sc, 2 * plo:2 * phi, 0:HD],
                    ps[:, 0:w].rearrange("p (h d) -> p h d", d=HD))

            def _emit_proj(w, dst, p, sqb, act_copy):
                sq = slice(sqb * SQB, (sqb + 1) * SQB)
                ps = ps_mm.tile([128, SQB], F32, tag="ps",
                                name=f"pj{p}_{sqb}")
                for c in range(KC):
                    nc.tensor.matmul(
                        ps[:, :], w[:, c, p * 128:(p + 1) * 128], xT[:, c, sq],
                        start=(c == 0), stop=(c == KC - 1))
                if act_copy:
                    nc.scalar.copy(dst[:, p, sq], ps[:, :])
                else:
                    nc.vector.tensor_copy(dst[:, p, sq], ps[:, :])

            def emit_q_proj(p, sqb, act_copy=False):
                _emit_proj(wq, qT, p, sqb, act_copy)

            def emit_k_proj(p, sqb, act_copy=False):
                _emit_proj(wk, kT, p, sqb, act_copy)

            # minimal preamble: q(0,0) + k(0,0) gate the first scores; the
            # remaining k(0,*) are dripped early inside the stream (scores
            # chunk 4j only needs kT up to sq-block j).
            emit_q_proj(0, 0)
            emit_k_proj(0, 0)

            # ---- global chunk stream -----------------------------------
            # All 16 blocks' score/exp chunks form one stream of 256 steps;
            # ctx matmuls trail by `lag` steps, so block boundaries pipeline
            # (the previous block's ctx tail and epilogue overlap the next
            # block's score/exp head).  PE drip work (projections, v
            # slices, out-proj) is scheduled at explicit stream indices.
            lag = ctx_lag
            fill = {}

            def drip(gi, thunk):
                fill.setdefault(gi, []).append(thunk)

            # k for pair 0, sq-blocks 1-3: needed by scores chunk 4j
            drip(1, lambda: emit_k_proj(0, 1))
            drip(4, lambda: emit_k_proj(0, 2))
            drip(7, lambda: emit_k_proj(0, 3))
            # v slices, pair 0 (needed at ctx(sc) = stream sc+lag; start at
            # +3 so the wv DMA has landed before the first slice)
            for sc in range(NSK):
                drip(sc + 3, lambda sc=sc: emit_v_slice(sc, 0, 1))
            # v slices for pairs 1-3 share one xT stationary load per chunk
            for sc in range(NSK):
                drip(18 + 2 * sc, lambda sc=sc: emit_v_slice(sc, 1, 4))
            # q for pair 0, sq-blocks 1-3: needed at stream 16j
            drip(11, lambda: emit_q_proj(0, 1))
            drip(26, lambda: emit_q_proj(0, 2))
            drip(42, lambda: emit_q_proj(0, 3))
            # q/k for pair 1: k needed by 64+4j, q(1,j) by 64+16j
            drip(46, lambda: emit_q_proj(1, 0))
            drip(50, lambda: emit_k_proj(1, 0))
            drip(54, lambda: emit_k_proj(1, 1))
            drip(58, lambda: emit_k_proj(1, 2))
            drip(61, lambda: emit_k_proj(1, 3))
            drip(74, lambda: emit_q_proj(1, 1))
            drip(90, lambda: emit_q_proj(1, 2))
            drip(104, lambda: emit_q_proj(1, 3))
            # q/k for pair 2
            drip(108, lambda: emit_q_proj(2, 0))
            drip(112, lambda: emit_k_proj(2, 0))
            drip(116, lambda: emit_k_proj(2, 1))
            drip(120, lambda: emit_k_proj(2, 2))
            drip(124, lambda: emit_k_proj(2, 3))
            drip(138, lambda: emit_q_proj(2, 1))
            drip(154, lambda: emit_q_proj(2, 2))
            drip(168, lambda: emit_q_proj(2, 3))
            # q/k for pair 3 (psum->sbuf copies routed to the ACT engine:
            # these land in fast-exp blocks where ACT has freed slots and
            # the DVE is carrying the offloaded exps)
            drip(172, lambda: emit_q_proj(3, 0, act_copy=True))
            drip(176, lambda: emit_k_proj(3, 0, act_copy=True))
            drip(180, lambda: emit_k_proj(3, 1, act_copy=True))
            drip(184, lambda: emit_k_proj(3, 2, act_copy=True))
            drip(188, lambda: emit_k_proj(3, 3, act_copy=True))
            drip(202, lambda: emit_q_proj(3, 1, act_copy=True))
            drip(218, lambda: emit_q_proj(3, 2, act_copy=True))
            drip(232, lambda: emit_q_proj(3, 3, act_copy=True))

            def emit_outproj_sc2(sc2):
                # pp-outer so each ctxT stationary is loaded once for both
                # 512-wide output halves
                s2 = slice(sc2 * 128, (sc2 + 1) * 128)
                po0 = ps_mm.tile([128, 512], F32, tag="ps", name=f"op{sc2}_0")
                po1 = ps_mm.tile([128, 512], F32, tag="ps", name=f"op{sc2}_1")
                for pp in range(NPAIR):
                    nc.tensor.matmul(
                        po0[:, :], ctxT[:, pp, s2], wo[:, pp, 0:512],
                        start=(pp == 0), stop=(pp == NPAIR - 1))
                    nc.tensor.matmul(
                        po1[:, :], ctxT[:, pp, s2], wo[:, pp, 512:1024],
                        start=(pp == 0), stop=(pp == NPAIR - 1))
                for oh, po in ((0, po0), (1, po1)):
                    ot = outp.tile([128, 512], BF16, tag="ot")
                    nc.vector.tensor_copy(ot[:, :], po[:, :])
                    eng = nc.sync if oh == 0 else nc.scalar
                    eng.dma_start(
                        out=out_d[s2, oh * 512:(oh + 1) * 512], in_=ot[:, :])

            # out-proj: block (3, sqb)'s four 128-row slices, spread over
            # the stream right after that block's epilogue (the last
            # block's epilogue lands at step NBLK*NSK thanks to the lag
            # taper below)
            for sqb in range(NSQB):
                if sqb < NSQB - 1:
                    epi = 16 * (3 * NSQB + sqb) + NSK - 1 + lag
                else:
                    epi = NPAIR * NSQB * NSK
                for k2 in range(SQB // 128):
                    drip(epi + 1 + k2,
                         lambda sc2=sqb * (SQB // 128) + k2: emit_outproj_sc2(sc2))

            def epilogue(p, sqb, cA, cB):
                sq = slice(sqb * SQB, (sqb + 1) * SQB)
                # 1/denom, broadcast over 64 partitions, scale.  The fast NR
                # reciprocal miscomputes on partition-sliced inputs, so feed
                # it the FULL [65, SQB] ctx psum tile: rows 0-63 produce
                # garbage nothing reads; row 64 is the denominator row.
                dnr = dnrp.tile([65, 2 * SQB], F32, tag="dnr")
                nc.vector.reciprocal_approx_fast(
                    out=dnr[0:65, 0:SQB], in_=cA[0:65, :])
                nc.vector.reciprocal_approx_fast(
                    out=dnr[0:65, SQB:2 * SQB], in_=cB[0:65, :])
                bcA = ps_mm.tile([64, SQB], F32, tag="ps", name="bcA")
                nc.tensor.matmul(
                    bcA[:, :], ones[64:65, 0:64], dnr[64:65, 0:SQB],
                    start=True, stop=True, tile_position=(64, 0))
                bcB = ps_mm.tile([64, SQB], F32, tag="ps", name="bcB")
                nc.tensor.matmul(
                    bcB[:, :], ones[64:65, 0:64], dnr[64:65, SQB:2 * SQB],
                    start=True, stop=True, tile_position=(64, 0))
                bc = bcp.tile([64, 2 * SQB], F32, tag="bc")
                nc.vector.tensor_copy(bc[:, 0:SQB], bcA[:, :])
                nc.vector.tensor_copy(bc[:, SQB:2 * SQB], bcB[:, :])
                nc.vector.tensor_mul(
                    ctxT[0:64, p, sq], cA[0:64, :], bc[:, 0:SQB])
                tmb = tmbp.tile([64, SQB], BF16, tag="tmb")
                nc.vector.tensor_mul(tmb[:, :], cB[0:64, :], bc[:, SQB:2 * SQB])
                # partition shift (rows 0-63 -> 64-127) via DMA
                nc.sync.dma_start(out=ctxT[64:128, p, sq], in_=tmb[:, :])

            NBLK = NPAIR * NSQB
            exps = [None] * (NBLK * NSK)
            cur = {}  # block -> (cA, cB)

            def emit_ctx(gj):
                b, j = gj // NSK, gj % NSK
                p = b // NSQB
                if j == 0:
                    cur[b] = (ps_ctx.tile([HD + 1, SQB], F32, tag="cA",
                                          name=f"cA{b}"),
                              ps_ctx.tile([HD + 1, SQB], F32, tag="cB",
                                          name=f"cB{b}"))
                cA, cB = cur[b]
                exl = exps[gj]
                nc.tensor.matmul(
                    cA[:, :], vsb[:, j, 2 * p, :], exl[:, 0:SQB],
                    start=(j == 0), stop=(j == NSK - 1))
                nc.tensor.matmul(
                    cB[:, :], vsb[:, j, 2 * p + 1, :], exl[:, SQB:2 * SQB],
                    start=(j == 0), stop=(j == NSK - 1))
                if j == NSK - 1:
                    epilogue(p, b % NSQB, cA, cB)
                    del cur[b]

            last_gi = max(NBLK * NSK + lag - 1, max(fill))
            next_ctx = 0
            for gi in range(last_gi + 1):
                if gi < NBLK * NSK:
                    b, i = gi // NSK, gi % NSK
                    p, sqb = b // NSQB, b % NSQB
                    sq = slice(sqb * SQB, (sqb + 1) * SQB)
                    sk = slice(i * 128, (i + 1) * 128)
                    sc_ps = ps_sc.tile([128, 2 * SQB], F32, tag="sc")
                    # two heads packed in the PE rows (K=64 each)
                    nc.tensor.matmul(
                        sc_ps[:, 0:SQB], kT[0:64, p, sk], qT[0:64, p, sq],
                        start=True, stop=True, tile_position=(0, 0))
                    nc.tensor.matmul(
                        sc_ps[:, SQB:2 * SQB], kT[64:128, p, sk],
                        qT[64:128, p, sq],
                        start=True, stop=True, tile_position=(64, 0))
                    ex = expp.tile([128, 2 * SQB], BF16, tag="ex")
                    if i in fast_exp.get((p, sqb), ()):
                        # DVE fast-exp: Schraudolph in bf16 bit space
                        nc.vector.tensor_scalar(
                            out=ex[:, :].bitcast(I16), in0=sc_ps[:, :],
                            scalar1=FE_A, scalar2=maskf[:, i:i + 1],
                            op0=mybir.AluOpType.mult,
                            op1=mybir.AluOpType.add)
                    else:
                        nc.scalar.activation(
                            out=ex[:, :], in_=sc_ps[:, :],
                            func=mybir.ActivationFunctionType.Exp,
                            bias=mask[:, i:i + 1], scale=1.0)
                    exps[gi] = ex
                for thunk in fill.get(gi, ()):
                    thunk()
                # ctx trails by `lag`, tapering to 1 over the final steps so
                # the last epilogue + out-proj tail starts as early as
                # possible
                target = min(gi - min(lag, max(1, NBLK * NSK - gi)),
                             NBLK * NSK - 1)
                while next_ctx <= target:
                    emit_ctx(next_ctx)
                    next_ctx += 1

        if loop_n is None:
            emit()
        else:
            with tc.For_i(0, loop_n, 1):
                emit()

    nc.compile()
    return nc


def _prep_core_inputs(x, am, Wq, Aq, Bq, Wk, Wv, Av, Bv, Wo):
    """Host-side shard + layout prep. Returns the 8 per-core input dicts."""
    s = 1.0 / math.sqrt(HD)
    # fold LoRA into the dense weights: q/v = x @ (W + scaling*B@A).T
    WqE = (Wq + SCALING * (Bq @ Aq)) * s
    WvE = Wv + SCALING * (Bv @ Av)
    wqT = np.ascontiguousarray(WqE.T).astype(NPBF16)           # [D, D]
    wkT = np.ascontiguousarray(Wk.T).astype(NPBF16)
    wvT = np.ascontiguousarray(WvE.T).astype(NPBF16)
    woT = np.ascontiguousarray(Wo.T).astype(NPBF16)            # [D, D]
    in_maps = []
    for core in range(NCORES):
        b, hh = core // 2, core % 2
        cs = slice(hh * OC, (hh + 1) * OC)
        xT = np.ascontiguousarray(x[b].T).astype(NPBF16)       # [D, S]
        m = np.ascontiguousarray(
            am[b, 0, 0, :].astype(np.float32).reshape(NSK, 128).T)  # [128, NSK]
        mf = np.ascontiguousarray(m * FE_A + (16256.0 - FE_C))
        in_maps.append({
            "xT": xT,
            "wq": np.ascontiguousarray(wqT[:, cs]),
            "wk": np.ascontiguousarray(wkT[:, cs]),
            "wv": np.ascontiguousarray(wvT[:, cs]),
            "wo": np.ascontiguousarray(woT[cs, :]),
            "mask": m,
            "maskf": mf,
        })
    return in_maps


def kernel(_trace=False, _trace_kwargs=None, **inputs):
    x = np.asarray(inputs["hidden_states"], dtype=np.float32)
    am = np.asarray(inputs["attention_mask"], dtype=np.float32)
    Wq = np.asarray(inputs["Wq"], dtype=np.float32)
    bq = np.asarray(inputs["bq"], dtype=np.float32)
    Aq = np.asarray(inputs["Aq"], dtype=np.float32)
    Bq = np.asarray(inputs["Bq"], dtype=np.float32)
    Wk = np.asarray(inputs["Wk"], dtype=np.float32)
    bk = np.asarray(inputs["bk"], dtype=np.float32)
    Wv = np.asarray(inputs["Wv"], dtype=np.float32)
    bv = np.asarray(inputs["bv"], dtype=np.float32)
    Av = np.asarray(inputs["Av"], dtype=np.float32)
    Bv = np.asarray(inputs["Bv"], dtype=np.float32)
    Wo = np.asarray(inputs["Wo"], dtype=np.float32)
    bo = np.asarray(inputs["bo"], dtype=np.float32)

    # Projection biases are all-zero in this problem's regime (asserted so
    # a violated assumption fails loudly rather than silently returning
    # wrong results).
    assert not bq.any() and not bk.any() and not bv.any(), (
        "non-zero projection biases not supported by this kernel build")

    if "nc" not in _NC_CACHE:
        _NC_CACHE["nc"] = _build_nc()
    nc = _NC_CACHE["nc"]

    in_maps = _prep_core_inputs(x, am, Wq, Aq, Bq, Wk, Wv, Av, Bv, Wo)
    res = run_bass_kernel_spmd(
        nc, in_maps, core_ids=list(range(NCORES)), trace=_trace,
        trace_kwargs=_trace_kwargs or {})
    outs = res.results

    out = np.empty((B, S, D), dtype=np.float32)
    for b in range(B):
        out[b] = (outs[2 * b]["out"].astype(np.float32)
                  + outs[2 * b + 1]["out"].astype(np.float32) + bo)
    if _trace:
        return out, res
    return out
